# revision 2
# baseline (speedup 1.0000x reference)
"""Trainium2 Bass kernel for AngularSymmetryMod — v3.

Per core (2 molecules): partitions = (b:2, i:32, half:2) = 128, free = 264
packed (j<=k) pairs. out[b,i,l] = sum_pairs ang*rad*cut with the 40-value
parameter grid collapsed to 20 reductions (4 angular quadrant fields x 5
radial gaussians), assembled into 40 columns by the final matmul.

v3 structure (from v1/v2 trace analysis):
 - fp32 everywhere: this DVE build runs bf16 tensor_tensor at HALF fp32 rate
   (custom uop tables), so bf16 only lost precision.
 - Radial chain lives entirely on ScalarE: sq_r = Square(q*0.5*sqrt(eta) -
   sqrt(eta)*Rs) (5 ops) then ONE wide Exp over [P,1320]. Square+Exp share
   the exp_and_others table; only Sin switches tables (2 loads, first hidden
   in launch).
 - GpSimd (tensor_tensor only - no TensorScalarPtr support) computes q, den,
   cut and the y/z dot products from SBUF.
 - cut is folded into the radial weights (Wc5 = rad5 * cut, one wide op)
   during the load2 window -> off the post-sin tail.
 - Post-sin tail: Sin -> (1+-x)^2 -> 4th powers on ACT while DVE runs the
   20 fused multiply+accumulate reductions back-to-back.
 - DMA: 5 transfers with triggers spread over sync/scalar/gpsimd queues
   (each DIRECT2D costs ~0.65us serially per queue), ordered so the radial
   inputs (uff) land first and geo-z last.
"""

import sys
import numpy as np
import ml_dtypes

sys.path.insert(0, "/opt/trn_rl_repo")


def _to_bf16(a):
    return np.asarray(a, dtype=np.float32).astype(ml_dtypes.bfloat16)

from contextlib import ExitStack

import concourse.bass as bass
import concourse.tile as tile
from concourse import bacc, mybir
from concourse.bass_utils import run_bass_kernel_spmd

B, N, L = 16, 32, 40
NCORES = 8
B_LOC = B // NCORES  # 2
P = 128
NT = 264
NOFF = 248

BOHR = 0.52917721092
ITA = 1.12
RS_VALS = np.array([0.5, 1.17, 1.83, 2.5, 3.17]) / BOHR
NR, NM = 5, 4
SQ_ETA = float(np.sqrt(ITA))
TWO_PI = float(2.0 * np.pi)
RC = float(12582912.0)

F32 = mybir.dt.float32
OP = mybir.AluOpType
ACT = mybir.ActivationFunctionType


def _pair_index():
    pairs = [(j, k) for j in range(N) for k in range(j + 1, N)]  # 496
    halves = [pairs[0::2], pairs[1::2]]
    tri_j = np.zeros((2, NT), dtype=np.int64)
    tri_k = np.zeros((2, NT), dtype=np.int64)
    for h in range(2):
        for t, (j, k) in enumerate(halves[h]):
            tri_j[h, t], tri_k[h, t] = j, k
        for t2, j in enumerate(range(h * 16, (h + 1) * 16)):
            tri_j[h, NOFF + t2] = tri_k[h, NOFF + t2] = j
    return tri_j, tri_k


_TRI_J, _TRI_K = _pair_index()


def _build():
    nc = bacc.Bacc("TRN2", target_bir_lowering=False, debug=False)
    uu_d = nc.declare_dram_parameter("uu", [P, 2 * NT], F32, isOutput=False)
    ff_d = nc.declare_dram_parameter("ff", [P, 2 * NT], F32, isOutput=False)
    # pes: [4=(b,h)] rows; cols = bsel lhsT [0:128] then per coord c: cj-source
    # [264] and ck-source [264] in hi/lo bf16 pairs
    BF = mybir.dt.bfloat16
    pes_d = nc.declare_dram_parameter("pes", [4, 128 + 12 * NT], BF, isOutput=False)
    cc_d = nc.declare_dram_parameter("cc", [P, 67], F32, isOutput=False)
    out_d = nc.declare_dram_parameter("out", [B_LOC * N, L], F32, isOutput=True)

    with tile.TileContext(nc) as tc, ExitStack() as ctx:
        pool = ctx.enter_context(tc.tile_pool(name="sb", bufs=1))
        scr_pool = ctx.enter_context(tc.tile_pool(name="scr", bufs=4))
        psum = ctx.enter_context(tc.tile_pool(name="ps", bufs=1, space="PSUM"))

        # ---- input DMAs ----
        BF = mybir.dt.bfloat16
        uu = pool.tile([P, 2 * NT], F32, name="uu", tag="uu")
        ff = pool.tile([P, 2 * NT], F32, name="ff", tag="ff")
        pes = pool.tile([4, 128 + 12 * NT], BF, name="pes", tag="pes")
        cc = pool.tile([P, 67], F32, name="cc", tag="cc")
        nc.gpsimd.dma_start(pes[:], pes_d[:])
        nc.gpsimd.dma_start(cc[:], cc_d[:])
        nc.sync.dma_start(uu[:], uu_d[:])
        nc.sync.dma_start(ff[:], ff_d[:])
        # constant tiles (gpsimd memsets, after its DMA triggers)
        rsb = pool.tile([P, NR], F32, name="rsb", tag="rsb")
        for r in range(NR):
            nc.gpsimd.memset(rsb[:, r : r + 1], float(-SQ_ETA * RS_VALS[r]))
        wcol = pool.tile([P, NT], F32, name="wcol", tag="wcol")
        nc.gpsimd.memset(wcol[:, 0:NOFF], 0.25)
        nc.gpsimd.memset(wcol[:, NOFF:NT], 0.125)
        # (a) dummy first activation with a launch-time-ready input so the
        # exp_and_others ACT_TABLE_LOAD is hoisted into the launch window
        dmy = pool.tile([P, 1], F32, name="dmy", tag="dmy")
        nc.scalar.activation(dmy[:], rsb[:, 0:1], ACT.Square)
        uj = uu[:, 0:NT]
        uk = uu[:, NT : 2 * NT]
        fj = ff[:, 0:NT]
        fk = ff[:, NT : 2 * NT]
        cic = cc[:, 0:3]
        cst = cc[:, 3:67]

        # ---- geo expansion on the TensorEngine: cj/ck = bsel.T @ src ----
        # exact fp32 via hi/lo bf16 pairs accumulated in PSUM; one LDWEIGHTS
        bsel = pes[:, 0:128]
        geo_ps = []
        for c in range(3):
            for jk in range(2):
                t0 = 128 + (c * 4 + jk * 2) * NT
                g = psum.tile([P, NT], F32, name=f"gp{c}{jk}", tag=f"gp{c}{jk}")
                nc.tensor.matmul(g[:], bsel, pes[:, t0 : t0 + NT],
                                 start=True, stop=False)
                nc.tensor.matmul(g[:], bsel, pes[:, t0 + NT : t0 + 2 * NT],
                                 start=False, stop=True)
                geo_ps.append(g)

        # ---- q on DVE (gates the whole ACT radial chain); den/cut on GpSimd ----
        q = pool.tile([P, NT], F32, name="q", tag="q")
        den = pool.tile([P, NT], F32, name="den", tag="den")
        nc.vector.tensor_tensor(q[:], uj, uk, OP.add)
        nc.gpsimd.tensor_tensor(den[:], uj, uk, OP.mult)
        cutw = pool.tile([P, NT], F32, name="cutw", tag="cutw")
        cut = pool.tile([P, NT], F32, name="cut", tag="cut")
        nc.gpsimd.tensor_tensor(cutw[:], fj, fk, OP.mult)
        nc.gpsimd.tensor_tensor(cut[:], cutw[:], wcol[:], OP.mult)

        # ---- radial: hv on DVE, Square(3NT wide)+ACT-biased rest, wide Exp ----
        hv5 = pool.tile([P, 5 * NT], F32, name="hv5", tag="hv5")
        sq5 = pool.tile([P, 5 * NT], F32, name="sq5", tag="sq5")
        rad5 = pool.tile([P, 5 * NT], F32, name="rad5", tag="rad5")
        for r in range(NR):
            nc.vector.tensor_scalar(
                hv5[:, r * NT : (r + 1) * NT], q[:],
                0.5 * SQ_ETA, float(-SQ_ETA * RS_VALS[r]), OP.mult, OP.add)
        nc.scalar.activation(sq5[:], hv5[:], ACT.Square)
        nc.scalar.activation(rad5[:], sq5[:], ACT.Exp, scale=-1.0)

        # ---- DVE dot-product chain ----
        vjx = pool.tile([P, NT], F32, name="vjx", tag="vjx")
        vkx = pool.tile([P, NT], F32, name="vkx", tag="vkx")
        vjy = pool.tile([P, NT], F32, name="vjy", tag="vjy")
        vky = pool.tile([P, NT], F32, name="vky", tag="vky")
        vjz = pool.tile([P, NT], F32, name="vjz", tag="vjz")
        vkz = pool.tile([P, NT], F32, name="vkz", tag="vkz")
        nc.vector.tensor_scalar(vjx[:], geo_ps[0][:], cic[:, 0:1], None, OP.subtract)
        nc.vector.tensor_scalar(vkx[:], geo_ps[1][:], cic[:, 0:1], None, OP.subtract)
        nc.vector.tensor_scalar(vjy[:], geo_ps[2][:], cic[:, 1:2], None, OP.subtract)
        nc.vector.tensor_scalar(vky[:], geo_ps[3][:], cic[:, 1:2], None, OP.subtract)
        nc.vector.tensor_scalar(vjz[:], geo_ps[4][:], cic[:, 2:3], None, OP.subtract)
        nc.vector.tensor_scalar(vkz[:], geo_ps[5][:], cic[:, 2:3], None, OP.subtract)

        px = pool.tile([P, NT], F32, name="px", tag="px")
        py = pool.tile([P, NT], F32, name="py", tag="py")
        pz = pool.tile([P, NT], F32, name="pz", tag="pz")
        nc.vector.tensor_tensor(px[:], vjx[:], vkx[:], OP.mult)
        nc.vector.tensor_tensor(py[:], vjy[:], vky[:], OP.mult)
        nc.vector.tensor_tensor(pz[:], vjz[:], vkz[:], OP.mult)

        # denb = 2pi*(den + 1e-5); rden = 1/denb   (DVE, fits in gaps)
        denb = pool.tile([P, NT], F32, name="denb", tag="denb")
        rden = pool.tile([P, NT], F32, name="rden", tag="rden")
        nc.vector.tensor_scalar(denb[:], den[:], 1e-5, TWO_PI, OP.add, OP.mult)
        nc.vector.reciprocal_approx_fast(rden[:], denb[:])

        dot01 = pool.tile([P, NT], F32, name="dot01", tag="dot01")
        dot = pool.tile([P, NT], F32, name="dot", tag="dot")
        nc.vector.tensor_tensor(dot01[:], px[:], py[:], OP.add)
        nc.vector.tensor_tensor(dot[:], dot01[:], pz[:], OP.add)

        tp2 = pool.tile([P, 2 * NT], F32, name="tp2", tag="tp2")
        nc.vector.tensor_tensor(tp2[:, 0:NT], dot[:], rden[:], OP.mult)
        nc.vector.tensor_scalar(tp2[:, NT : 2 * NT], tp2[:, 0:NT], 0.25, None, OP.add)
        nf2 = pool.tile([P, 2 * NT], F32, name="nf2", tag="nf2")
        nc.vector.tensor_scalar(nf2[:], tp2[:], RC, RC, OP.add, OP.subtract)
        fr2 = pool.tile([P, 2 * NT], F32, name="fr2", tag="fr2")
        nc.vector.tensor_tensor(fr2[:], tp2[:], nf2[:], OP.subtract)

        # ---- Wc5 = rad5 * cut on DVE (runs during the LOAD2 window) ----
        Wc5 = pool.tile([P, 5 * NT], F32, name="Wc5", tag="Wc5")
        for r in range(NR):
            nc.vector.tensor_tensor(
                Wc5[:, r * NT : (r + 1) * NT], rad5[:, r * NT : (r + 1) * NT],
                cut[:], OP.mult)

        # ---- sin/cos + angular powers, split per trig half so the +s reduce
        # block starts while the cos half is still on ACT ----
        one = pool.tile([P, 1], F32, name="one", tag="one")
        nc.gpsimd.memset(one[:], 1.0)
        cs = pool.tile([P, 2 * NT], F32, name="cs", tag="cs")
        up = pool.tile([P, 2 * NT], F32, name="up", tag="up")
        ap = pool.tile([P, 2 * NT], F32, name="ap", tag="ap")
        um = pool.tile([P, 2 * NT], F32, name="um", tag="um")
        am = pool.tile([P, 2 * NT], F32, name="am", tag="am")
        H = [slice(0, NT), slice(NT, 2 * NT)]
        for h in H:
            nc.scalar.activation(cs[:, h], fr2[:, h], ACT.Sin, scale=TWO_PI)
            nc.scalar.activation(up[:, h], cs[:, h], ACT.Square, bias=one[:], scale=1.0)
            nc.scalar.activation(ap[:, h], up[:, h], ACT.Square)
        for h in H:
            nc.scalar.activation(um[:, h], cs[:, h], ACT.Square, bias=one[:], scale=-1.0)
            nc.scalar.activation(am[:, h], um[:, h], ACT.Square)

        # ---- 20 fused multiply+reduce on DVE; final matmuls overlap the train ----
        spart = pool.tile([P, 24], F32, name="spart", tag="spart")
        s2p = psum.tile([64, L], F32, name="s2p", tag="s2p")
        sp3 = spart[:, 0 : NR * NM].rearrange("p (r t) -> p r t", r=NR, t=NM)
        o3 = s2p[:].rearrange("n (g r t) -> n g r t", g=2, r=NR, t=NM)

        # (field, col-offset within [s|c], tcol): +s->1, +c->0, -s->3, -c->2
        def reduce_block(field, off, tcol):
            for r in range(NR):
                scr = scr_pool.tile([P, NT], F32, name=f"scr{r}{tcol}", tag="scr")
                nc.vector.scalar_tensor_tensor(
                    scr[:], Wc5[:, r * NT : (r + 1) * NT], 0.0,
                    field[:, off : off + NT], OP.bypass, OP.mult,
                    accum_out=spart[:, r * NM + tcol : r * NM + tcol + 1])

        reduce_block(ap, 0, 1)    # +s
        reduce_block(ap, NT, 0)   # +c
        # lam=+1 cols t in {0,1} and lam=-1 cols t in {2,3} need only tcol 0,1
        nc.tensor.matmul(o3[:, 0, :, 0:2], cst, sp3[:, :, 0:2])
        nc.tensor.matmul(o3[:, 1, :, 2:4], cst, sp3[:, :, 0:2])
        reduce_block(am, 0, 3)    # -s
        reduce_block(am, NT, 2)   # -c
        nc.tensor.matmul(o3[:, 0, :, 2:4], cst, sp3[:, :, 2:4])
        nc.tensor.matmul(o3[:, 1, :, 0:2], cst, sp3[:, :, 2:4])
        s2s = pool.tile([64, L], F32, name="s2s", tag="s2s")
        nc.vector.tensor_copy(s2s[:], s2p[:])
        nc.sync.dma_start(out_d[:], s2s[:])

    nc.compile()
    return nc


def _ensure_ntff_hook():
    import types

    try:
        from antenv.axon_hooks import get_axon_ntff_profile_hook
        if get_axon_ntff_profile_hook() is not None:
            return
        have_mod = True
    except ImportError:
        have_mod = False
    try:
        if "/root/.axon_site" not in sys.path:
            sys.path.insert(0, "/root/.axon_site")
        from trn_agent_boot.trn_boot import _ntff_profile_via_ctypes

        hook = _ntff_profile_via_ctypes("/opt/axon/libaxon_pjrt.so")
        if hook is None:
            return
    except Exception:
        return
    if have_mod:
        from antenv import axon_hooks
        axon_hooks.set_axon_ntff_profile_hook(hook)
    else:
        m = types.ModuleType("antenv.axon_hooks")
        _h = [hook]
        m.get_axon_ntff_profile_hook = lambda: _h[0]
        m.set_axon_ntff_profile_hook = lambda h: _h.__setitem__(0, h)
        import antenv
        antenv.axon_hooks = m
        sys.modules["antenv.axon_hooks"] = m


_NC = None


def _get_nc():
    global _NC
    if _NC is None:
        _NC = _build()
    return _NC


_CST = None


def _const_blob():
    global _CST
    if _CST is None:
        _CST = np.repeat(np.eye(64, dtype=np.float32), 2, axis=0)
    return _CST


def _host_pack(d_cutoff, d, atom_coordinates):
    """Pure gather/replication of raw inputs into the per-core layouts."""
    d_cutoff = np.ascontiguousarray(d_cutoff, dtype=np.float32)
    d = np.ascontiguousarray(d, dtype=np.float32)
    coords = np.ascontiguousarray(atom_coordinates, dtype=np.float32)

    p = np.arange(P)
    b_of_p = p // (N * 2)
    i_of_p = (p // 2) % N
    half = p % 2
    jt = _TRI_J[half]  # [P, NT]
    kt = _TRI_K[half]

    in_maps = []
    for c in range(NCORES):
        cd = coords[c * B_LOC : (c + 1) * B_LOC]
        dd = d[c * B_LOC : (c + 1) * B_LOC]
        fc = d_cutoff[c * B_LOC : (c + 1) * B_LOC]
        uu = np.empty((P, 2 * NT), dtype=np.float32)
        uu[:, 0:NT] = dd[b_of_p[:, None], i_of_p[:, None], jt]
        uu[:, NT : 2 * NT] = dd[b_of_p[:, None], i_of_p[:, None], kt]
        ffb = np.empty((P, 2 * NT), dtype=np.float32)
        ffb[:, 0:NT] = fc[b_of_p[:, None], i_of_p[:, None], jt]
        ffb[:, NT : 2 * NT] = fc[b_of_p[:, None], i_of_p[:, None], kt]
        # pes: bsel lhsT + per-(b,h) cj/ck source rows, hi/lo bf16 split
        pes = np.zeros((4, 128 + 12 * NT), dtype=np.float32)
        row_of_p = b_of_p * 2 + half  # [P]
        pes[row_of_p, np.arange(P)] = 1.0
        for cc in range(3):
            for bb in range(B_LOC):
                for h in range(2):
                    cj_src = cd[bb, _TRI_J[h], cc]  # [NT]
                    ck_src = cd[bb, _TRI_K[h], cc]
                    base = 128 + cc * 4 * NT
                    row = bb * 2 + h
                    for off, src in ((0, cj_src), (2 * NT, ck_src)):
                        hi16 = _to_bf16(src)
                        lo16 = _to_bf16(src - hi16.astype(np.float32))
                        pes[row, base + off : base + off + NT] = hi16.astype(np.float32)
                        pes[row, base + off + NT : base + off + 2 * NT] = lo16.astype(
                            np.float32
                        )
        pes_bf = _to_bf16(pes)
        ccb = np.empty((P, 67), dtype=np.float32)
        ccb[:, 0:3] = cd[b_of_p, i_of_p]
        ccb[:, 3:67] = _const_blob()
        in_maps.append({"uu": uu, "ff": ffb, "pes": pes_bf, "cc": ccb})
    return in_maps


def kernel(d_cutoff, d, atom_coordinates, _trace=False):
    if _trace:
        _ensure_ntff_hook()
    nc = _get_nc()
    in_maps = _host_pack(d_cutoff, d, atom_coordinates)
    res = run_bass_kernel_spmd(nc, in_maps, core_ids=list(range(NCORES)), trace=_trace)
    out = np.concatenate(
        [res.results[c]["out"].reshape(B_LOC, N, L) for c in range(NCORES)], axis=0
    ).astype(np.float32)
    if _trace:
        kernel._last_results = res
    return out


# revision 3
# speedup vs baseline: 1.0935x; 1.0935x over previous
"""Trainium2 Bass kernel for AngularSymmetryMod — v3.

Per core (2 molecules): partitions = (b:2, i:32, half:2) = 128, free = 264
packed (j<=k) pairs. out[b,i,l] = sum_pairs ang*rad*cut with the 40-value
parameter grid collapsed to 20 reductions (4 angular quadrant fields x 5
radial gaussians), assembled into 40 columns by the final matmul.

v3 structure (from v1/v2 trace analysis):
 - fp32 everywhere: this DVE build runs bf16 tensor_tensor at HALF fp32 rate
   (custom uop tables), so bf16 only lost precision.
 - Radial chain lives entirely on ScalarE: sq_r = Square(q*0.5*sqrt(eta) -
   sqrt(eta)*Rs) (5 ops) then ONE wide Exp over [P,1320]. Square+Exp share
   the exp_and_others table; only Sin switches tables (2 loads, first hidden
   in launch).
 - GpSimd (tensor_tensor only - no TensorScalarPtr support) computes q, den,
   cut and the y/z dot products from SBUF.
 - cut is folded into the radial weights (Wc5 = rad5 * cut, one wide op)
   during the load2 window -> off the post-sin tail.
 - Post-sin tail: Sin -> (1+-x)^2 -> 4th powers on ACT while DVE runs the
   20 fused multiply+accumulate reductions back-to-back.
 - DMA: 5 transfers with triggers spread over sync/scalar/gpsimd queues
   (each DIRECT2D costs ~0.65us serially per queue), ordered so the radial
   inputs (uff) land first and geo-z last.
"""

import sys
import numpy as np
import ml_dtypes

sys.path.insert(0, "/opt/trn_rl_repo")


def _to_bf16(a):
    return np.asarray(a, dtype=np.float32).astype(ml_dtypes.bfloat16)

from contextlib import ExitStack

import concourse.bass as bass
import concourse.tile as tile
from concourse import bacc, mybir
from concourse.bass_utils import run_bass_kernel_spmd

B, N, L = 16, 32, 40
NCORES = 8
B_LOC = B // NCORES  # 2
P = 128
NT = 264
NOFF = 248

BOHR = 0.52917721092
ITA = 1.12
RS_VALS = np.array([0.5, 1.17, 1.83, 2.5, 3.17]) / BOHR
NR, NM = 5, 4
SQ_ETA = float(np.sqrt(ITA))
TWO_PI = float(2.0 * np.pi)
RC = float(12582912.0)

F32 = mybir.dt.float32
OP = mybir.AluOpType
ACT = mybir.ActivationFunctionType


def _pair_index():
    pairs = [(j, k) for j in range(N) for k in range(j + 1, N)]  # 496
    halves = [pairs[0::2], pairs[1::2]]
    tri_j = np.zeros((2, NT), dtype=np.int64)
    tri_k = np.zeros((2, NT), dtype=np.int64)
    for h in range(2):
        for t, (j, k) in enumerate(halves[h]):
            tri_j[h, t], tri_k[h, t] = j, k
        for t2, j in enumerate(range(h * 16, (h + 1) * 16)):
            tri_j[h, NOFF + t2] = tri_k[h, NOFF + t2] = j
    return tri_j, tri_k


_TRI_J, _TRI_K = _pair_index()


def _build():
    nc = bacc.Bacc("TRN2", target_bir_lowering=False, debug=False)
    uu_d = nc.declare_dram_parameter("uu", [P, 2 * NT], F32, isOutput=False)
    ff_d = nc.declare_dram_parameter("ff", [P, 2 * NT], F32, isOutput=False)
    # pes: [4=(b,h)] rows; cols = bsel lhsT [0:128] then per coord c: cj-source
    # [264] and ck-source [264] in hi/lo bf16 pairs
    BF = mybir.dt.bfloat16
    pes_d = nc.declare_dram_parameter("pes", [4, 128 + 12 * NT], BF, isOutput=False)
    cc_d = nc.declare_dram_parameter("cc", [P, 67], F32, isOutput=False)
    out_d = nc.declare_dram_parameter("out", [B_LOC * N, L], F32, isOutput=True)

    with tile.TileContext(nc) as tc, ExitStack() as ctx:
        pool = ctx.enter_context(tc.tile_pool(name="sb", bufs=1))
        scr_pool = ctx.enter_context(tc.tile_pool(name="scr", bufs=4))
        psum = ctx.enter_context(tc.tile_pool(name="ps", bufs=1, space="PSUM"))

        # ---- input DMAs ----
        BF = mybir.dt.bfloat16
        uu = pool.tile([P, 2 * NT], F32, name="uu", tag="uu")
        ff = pool.tile([P, 2 * NT], F32, name="ff", tag="ff")
        pes = pool.tile([4, 128 + 12 * NT], BF, name="pes", tag="pes")
        cc = pool.tile([P, 67], F32, name="cc", tag="cc")
        nc.sync.dma_start(pes[:], pes_d[:])
        nc.sync.dma_start(uu[:], uu_d[:])
        nc.sync.dma_start(ff[:], ff_d[:])
        nc.gpsimd.dma_start(cc[:], cc_d[:])
        # constant tiles (gpsimd memsets, after its DMA triggers)
        rsb = pool.tile([P, NR], F32, name="rsb", tag="rsb")
        for r in range(NR):
            nc.gpsimd.memset(rsb[:, r : r + 1], float(-SQ_ETA * RS_VALS[r]))
        wcol = pool.tile([P, NT], F32, name="wcol", tag="wcol")
        nc.gpsimd.memset(wcol[:, 0:NOFF], 0.25)
        nc.gpsimd.memset(wcol[:, NOFF:NT], 0.125)
        # (a) dummy first activation with a launch-time-ready input so the
        # exp_and_others ACT_TABLE_LOAD is hoisted into the launch window
        dmy = pool.tile([P, 1], F32, name="dmy", tag="dmy")
        nc.scalar.activation(dmy[:], rsb[:, 0:1], ACT.Square)
        uj = uu[:, 0:NT]
        uk = uu[:, NT : 2 * NT]
        fj = ff[:, 0:NT]
        fk = ff[:, NT : 2 * NT]
        cic = cc[:, 0:3]
        cst = cc[:, 3:67]

        # ---- geo expansion on the TensorEngine: cj/ck = bsel.T @ src ----
        # exact fp32 via hi/lo bf16 pairs accumulated in PSUM; one LDWEIGHTS
        bsel = pes[:, 0:128]
        geo_ps = []
        for c in range(3):
            for jk in range(2):
                t0 = 128 + (c * 4 + jk * 2) * NT
                g = psum.tile([P, NT], F32, name=f"gp{c}{jk}", tag=f"gp{c}{jk}")
                nc.tensor.matmul(g[:], bsel, pes[:, t0 : t0 + NT],
                                 start=True, stop=False)
                nc.tensor.matmul(g[:], bsel, pes[:, t0 + NT : t0 + 2 * NT],
                                 start=False, stop=True)
                geo_ps.append(g)

        # ---- q on DVE (gates the whole ACT radial chain); den/cut on GpSimd ----
        q = pool.tile([P, NT], F32, name="q", tag="q")
        den = pool.tile([P, NT], F32, name="den", tag="den")
        nc.vector.tensor_tensor(q[:], uj, uk, OP.add)
        nc.gpsimd.tensor_tensor(den[:], uj, uk, OP.mult)
        cutw = pool.tile([P, NT], F32, name="cutw", tag="cutw")
        cut = pool.tile([P, NT], F32, name="cut", tag="cut")
        nc.gpsimd.tensor_tensor(cutw[:], fj, fk, OP.mult)
        nc.gpsimd.tensor_tensor(cut[:], cutw[:], wcol[:], OP.mult)

        # ---- radial: hv on DVE, Square(3NT wide)+ACT-biased rest, wide Exp ----
        hv5 = pool.tile([P, 5 * NT], F32, name="hv5", tag="hv5")
        sq5 = pool.tile([P, 5 * NT], F32, name="sq5", tag="sq5")
        rad5 = pool.tile([P, 5 * NT], F32, name="rad5", tag="rad5")
        for r in range(NR):
            nc.vector.tensor_scalar(
                hv5[:, r * NT : (r + 1) * NT], q[:],
                0.5 * SQ_ETA, float(-SQ_ETA * RS_VALS[r]), OP.mult, OP.add)
        nc.scalar.activation(sq5[:], hv5[:], ACT.Square)
        nc.scalar.activation(rad5[:], sq5[:], ACT.Exp, scale=-1.0)

        # ---- DVE dot-product chain ----
        vjx = pool.tile([P, NT], F32, name="vjx", tag="vjx")
        vkx = pool.tile([P, NT], F32, name="vkx", tag="vkx")
        vjy = pool.tile([P, NT], F32, name="vjy", tag="vjy")
        vky = pool.tile([P, NT], F32, name="vky", tag="vky")
        vjz = pool.tile([P, NT], F32, name="vjz", tag="vjz")
        vkz = pool.tile([P, NT], F32, name="vkz", tag="vkz")
        nc.vector.tensor_scalar(vjx[:], geo_ps[0][:], cic[:, 0:1], None, OP.subtract)
        nc.vector.tensor_scalar(vkx[:], geo_ps[1][:], cic[:, 0:1], None, OP.subtract)
        nc.vector.tensor_scalar(vjy[:], geo_ps[2][:], cic[:, 1:2], None, OP.subtract)
        nc.vector.tensor_scalar(vky[:], geo_ps[3][:], cic[:, 1:2], None, OP.subtract)
        nc.vector.tensor_scalar(vjz[:], geo_ps[4][:], cic[:, 2:3], None, OP.subtract)
        nc.vector.tensor_scalar(vkz[:], geo_ps[5][:], cic[:, 2:3], None, OP.subtract)

        px = pool.tile([P, NT], F32, name="px", tag="px")
        py = pool.tile([P, NT], F32, name="py", tag="py")
        pz = pool.tile([P, NT], F32, name="pz", tag="pz")
        nc.vector.tensor_tensor(px[:], vjx[:], vkx[:], OP.mult)
        nc.vector.tensor_tensor(py[:], vjy[:], vky[:], OP.mult)
        nc.vector.tensor_tensor(pz[:], vjz[:], vkz[:], OP.mult)

        # denb = 2pi*(den + 1e-5); rden = 1/denb   (DVE, fits in gaps)
        denb = pool.tile([P, NT], F32, name="denb", tag="denb")
        rden = pool.tile([P, NT], F32, name="rden", tag="rden")
        nc.vector.tensor_scalar(denb[:], den[:], 1e-5, TWO_PI, OP.add, OP.mult)
        nc.vector.reciprocal_approx_fast(rden[:], denb[:])

        dot01 = pool.tile([P, NT], F32, name="dot01", tag="dot01")
        dot = pool.tile([P, NT], F32, name="dot", tag="dot")
        nc.vector.tensor_tensor(dot01[:], px[:], py[:], OP.add)
        nc.vector.tensor_tensor(dot[:], dot01[:], pz[:], OP.add)

        tp2 = pool.tile([P, 2 * NT], F32, name="tp2", tag="tp2")
        nc.vector.tensor_tensor(tp2[:, 0:NT], dot[:], rden[:], OP.mult)
        nc.vector.tensor_scalar(tp2[:, NT : 2 * NT], tp2[:, 0:NT], 0.25, None, OP.add)
        nf2 = pool.tile([P, 2 * NT], F32, name="nf2", tag="nf2")
        nc.vector.tensor_scalar(nf2[:], tp2[:], RC, RC, OP.add, OP.subtract)
        fr2 = pool.tile([P, 2 * NT], F32, name="fr2", tag="fr2")
        nc.vector.tensor_tensor(fr2[:], tp2[:], nf2[:], OP.subtract)

        # ---- Wc5 = rad5 * cut on DVE (runs during the LOAD2 window) ----
        Wc5 = pool.tile([P, 5 * NT], F32, name="Wc5", tag="Wc5")
        for r in range(NR):
            nc.vector.tensor_tensor(
                Wc5[:, r * NT : (r + 1) * NT], rad5[:, r * NT : (r + 1) * NT],
                cut[:], OP.mult)

        # ---- sin/cos + angular powers, split per trig half so the +s reduce
        # block starts while the cos half is still on ACT ----
        one = pool.tile([P, 1], F32, name="one", tag="one")
        nc.gpsimd.memset(one[:], 1.0)
        cs = pool.tile([P, 2 * NT], F32, name="cs", tag="cs")
        up = pool.tile([P, 2 * NT], F32, name="up", tag="up")
        ap = pool.tile([P, 2 * NT], F32, name="ap", tag="ap")
        um = pool.tile([P, 2 * NT], F32, name="um", tag="um")
        am = pool.tile([P, 2 * NT], F32, name="am", tag="am")
        H = [slice(0, NT), slice(NT, 2 * NT)]
        for h in H:
            nc.scalar.activation(cs[:, h], fr2[:, h], ACT.Sin, scale=TWO_PI)
            nc.scalar.activation(up[:, h], cs[:, h], ACT.Square, bias=one[:], scale=1.0)
            nc.scalar.activation(ap[:, h], up[:, h], ACT.Square)
        for h in H:
            nc.scalar.activation(um[:, h], cs[:, h], ACT.Square, bias=one[:], scale=-1.0)
            nc.scalar.activation(am[:, h], um[:, h], ACT.Square)

        # ---- 20 fused multiply+reduce on DVE; final matmuls overlap the train ----
        spart = pool.tile([P, 24], F32, name="spart", tag="spart")
        s2p = psum.tile([64, L], F32, name="s2p", tag="s2p")
        sp3 = spart[:, 0 : NR * NM].rearrange("p (r t) -> p r t", r=NR, t=NM)
        o3 = s2p[:].rearrange("n (g r t) -> n g r t", g=2, r=NR, t=NM)

        # (field, col-offset within [s|c], tcol): +s->1, +c->0, -s->3, -c->2
        def reduce_block(field, off, tcol):
            for r in range(NR):
                scr = scr_pool.tile([P, NT], F32, name=f"scr{r}{tcol}", tag="scr")
                nc.vector.scalar_tensor_tensor(
                    scr[:], Wc5[:, r * NT : (r + 1) * NT], 0.0,
                    field[:, off : off + NT], OP.bypass, OP.mult,
                    accum_out=spart[:, r * NM + tcol : r * NM + tcol + 1])

        reduce_block(ap, 0, 1)    # +s
        reduce_block(ap, NT, 0)   # +c
        # lam=+1 cols t in {0,1} and lam=-1 cols t in {2,3} need only tcol 0,1
        nc.tensor.matmul(o3[:, 0, :, 0:2], cst, sp3[:, :, 0:2])
        nc.tensor.matmul(o3[:, 1, :, 2:4], cst, sp3[:, :, 0:2])
        reduce_block(am, 0, 3)    # -s
        reduce_block(am, NT, 2)   # -c
        nc.tensor.matmul(o3[:, 0, :, 2:4], cst, sp3[:, :, 2:4])
        nc.tensor.matmul(o3[:, 1, :, 0:2], cst, sp3[:, :, 2:4])
        s2s = pool.tile([64, L], F32, name="s2s", tag="s2s")
        nc.vector.tensor_copy(s2s[:], s2p[:])
        nc.sync.dma_start(out_d[:], s2s[:])

    nc.compile()
    return nc


def _ensure_ntff_hook():
    import types

    try:
        from antenv.axon_hooks import get_axon_ntff_profile_hook
        if get_axon_ntff_profile_hook() is not None:
            return
        have_mod = True
    except ImportError:
        have_mod = False
    try:
        if "/root/.axon_site" not in sys.path:
            sys.path.insert(0, "/root/.axon_site")
        from trn_agent_boot.trn_boot import _ntff_profile_via_ctypes

        hook = _ntff_profile_via_ctypes("/opt/axon/libaxon_pjrt.so")
        if hook is None:
            return
    except Exception:
        return
    if have_mod:
        from antenv import axon_hooks
        axon_hooks.set_axon_ntff_profile_hook(hook)
    else:
        m = types.ModuleType("antenv.axon_hooks")
        _h = [hook]
        m.get_axon_ntff_profile_hook = lambda: _h[0]
        m.set_axon_ntff_profile_hook = lambda h: _h.__setitem__(0, h)
        import antenv
        antenv.axon_hooks = m
        sys.modules["antenv.axon_hooks"] = m


_NC = None


def _get_nc():
    global _NC
    if _NC is None:
        _NC = _build()
    return _NC


_CST = None


def _const_blob():
    global _CST
    if _CST is None:
        _CST = np.repeat(np.eye(64, dtype=np.float32), 2, axis=0)
    return _CST


def _host_pack(d_cutoff, d, atom_coordinates):
    """Pure gather/replication of raw inputs into the per-core layouts."""
    d_cutoff = np.ascontiguousarray(d_cutoff, dtype=np.float32)
    d = np.ascontiguousarray(d, dtype=np.float32)
    coords = np.ascontiguousarray(atom_coordinates, dtype=np.float32)

    p = np.arange(P)
    b_of_p = p // (N * 2)
    i_of_p = (p // 2) % N
    half = p % 2
    jt = _TRI_J[half]  # [P, NT]
    kt = _TRI_K[half]

    in_maps = []
    for c in range(NCORES):
        cd = coords[c * B_LOC : (c + 1) * B_LOC]
        dd = d[c * B_LOC : (c + 1) * B_LOC]
        fc = d_cutoff[c * B_LOC : (c + 1) * B_LOC]
        uu = np.empty((P, 2 * NT), dtype=np.float32)
        uu[:, 0:NT] = dd[b_of_p[:, None], i_of_p[:, None], jt]
        uu[:, NT : 2 * NT] = dd[b_of_p[:, None], i_of_p[:, None], kt]
        ffb = np.empty((P, 2 * NT), dtype=np.float32)
        ffb[:, 0:NT] = fc[b_of_p[:, None], i_of_p[:, None], jt]
        ffb[:, NT : 2 * NT] = fc[b_of_p[:, None], i_of_p[:, None], kt]
        # pes: bsel lhsT + per-(b,h) cj/ck source rows, hi/lo bf16 split
        pes = np.zeros((4, 128 + 12 * NT), dtype=np.float32)
        row_of_p = b_of_p * 2 + half  # [P]
        pes[row_of_p, np.arange(P)] = 1.0
        for cc in range(3):
            for bb in range(B_LOC):
                for h in range(2):
                    cj_src = cd[bb, _TRI_J[h], cc]  # [NT]
                    ck_src = cd[bb, _TRI_K[h], cc]
                    base = 128 + cc * 4 * NT
                    row = bb * 2 + h
                    for off, src in ((0, cj_src), (2 * NT, ck_src)):
                        hi16 = _to_bf16(src)
                        lo16 = _to_bf16(src - hi16.astype(np.float32))
                        pes[row, base + off : base + off + NT] = hi16.astype(np.float32)
                        pes[row, base + off + NT : base + off + 2 * NT] = lo16.astype(
                            np.float32
                        )
        pes_bf = _to_bf16(pes)
        ccb = np.empty((P, 67), dtype=np.float32)
        ccb[:, 0:3] = cd[b_of_p, i_of_p]
        ccb[:, 3:67] = _const_blob()
        in_maps.append({"uu": uu, "ff": ffb, "pes": pes_bf, "cc": ccb})
    return in_maps


def kernel(d_cutoff, d, atom_coordinates, _trace=False):
    if _trace:
        _ensure_ntff_hook()
    nc = _get_nc()
    in_maps = _host_pack(d_cutoff, d, atom_coordinates)
    res = run_bass_kernel_spmd(nc, in_maps, core_ids=list(range(NCORES)), trace=_trace)
    out = np.concatenate(
        [res.results[c]["out"].reshape(B_LOC, N, L) for c in range(NCORES)], axis=0
    ).astype(np.float32)
    if _trace:
        kernel._last_results = res
    return out
